# revision 48
# baseline (speedup 1.0000x reference)
"""Trainium2 Bass kernel for nn_Denoiser_73598559584966.

Full-sequence self-attention (Q=K=V, no scaling) over x: [4, 16, 16, 16, 64]
  t = x.reshape(B, 4096, 64); out = softmax(t @ t^T) @ t

Sharding: 8 cores = 4 batches x 2 query-halves. Each core: 2048 queries
vs the full 4096 keys/values of its batch. No collectives.

Symmetric Gaussian-kernel form: with g_i = |t_i|^2/2,
  P'_ij = exp(t_i.t_j - g_i - g_j) = exp(-|t_i - t_j|^2/2) <= 1
is exactly symmetric (Q=K), so for key tiles inside the core's own
query range (host permutes keys so these are always kt 0..15) the
mirror P' tiles come from 6 batched DMA-XBAR transposes of computed
[128, 2048] regions: ~19% of the exp (ScalarE, a bottleneck engine)
and of the QK matmuls vanish.
The per-key factor e^{g_i - G} folds into bf16 V rows on host; the
per-query factor cancels in the final division. bf16 P/V have enough
range that no clipping/underflow handling is needed (rel err ~2.4e-3).

Device per core, chunked over 512-query column blocks:
  S'_kt[128, 512] = (k_kt | 1 | -g_k)^T (q | -g_q | 1)  fp16, K=66
  P' = exp(S')     ScalarE, fp32 PSUM -> bf16 SBUF, <=3 key tiles/ACT
    self strips (kt 0..15, kt >= 4ch) -> persistent Pbig (strided AP);
    one dma_start(transpose=True) per strip with kt >= 4(ch+1) writes
    the 4 mirror blocks of a later chunk's tiles (block-major 3D out);
    far strips (kt 16..31) -> transient pool
  O^T[65, 512] += (w.V_kt | w)^T P'_kt   bf16, K=128; row 64 = denom
Host epilogue divides rows 0..63 by row 64 and transposes per shard.
"""
import os
import numpy as np

B_, D_, H_, W_, C_ = 4, 16, 16, 16, 64
NTOK = D_ * H_ * W_          # 4096 tokens per batch
NQ = NTOK // 2               # 2048 queries per core
NCORES = 8
NKT = NTOK // 128            # 32 key tiles
NSELF = 16                   # key tiles covering this core's own queries
NCH = 4                      # query chunks per core
CHW = NQ // NCH              # 512 queries per chunk
NG = 4                       # DMA groups over key tiles
GKT = NKT // NG              # 8 key tiles per group

_CACHE = {}


def _units(k0, k1):
    """Split kt range [k0, k1) into groups of <=3."""
    out = []
    while k0 < k1:
        n = min(3, k1 - k0)
        out.append((k0, n))
        k0 += n
    return out


def _build_nc(sym=True):
    import concourse.bacc as bacc
    import concourse.mybir as mybir
    from concourse.tile import TileContext

    f32 = mybir.dt.float32
    f16 = mybir.dt.float16
    bf16 = mybir.dt.bfloat16
    EXP = mybir.ActivationFunctionType.Exp
    nc = bacc.Bacc("TRN2", target_bir_lowering=False, debug=False)

    q2 = nc.dram_tensor("q2", [66, NQ], f16, kind="ExternalInput")
    k2 = nc.dram_tensor("k2", [66, NTOK], f16, kind="ExternalInput")
    vpk = nc.dram_tensor("vpk", [128, NKT * 65], bf16, kind="ExternalInput")
    out = nc.dram_tensor("out", [65, NQ], f32, kind="ExternalOutput")

    with TileContext(nc) as tc:
        with (
            tc.tile_pool(name="const", bufs=1) as const,
            tc.tile_pool(name="pp", bufs=4) as pp,
            tc.tile_pool(name="sbo", bufs=2) as sbo,
            tc.tile_pool(name="ps_s", bufs=2, space="PSUM") as ps_s,
            tc.tile_pool(name="ps_o", bufs=2, space="PSUM") as ps_o,
        ):
            # ---- input DMAs (chunk-0 operands first) ----
            q2_t = const.tile([66, NQ], f16, tag="q2")
            k2_t = const.tile([66, NTOK], f16, tag="k2")
            vpk_t = const.tile([128, NKT * 65], bf16, tag="vpk")
            # split input DMAs across both HWDGE queues: Sync carries the
            # first half, the (still idle) Scalar queue carries the second
            nc.sync.dma_start(out=q2_t[:, 0:CHW], in_=q2[:, 0:CHW])
            for g in range(2):
                ks = slice(g * GKT * 128, (g + 1) * GKT * 128)
                nc.sync.dma_start(out=k2_t[:, ks], in_=k2[:, ks])
                vs = slice(g * GKT * 65, (g + 1) * GKT * 65)
                nc.sync.dma_start(out=vpk_t[:, vs], in_=vpk[:, vs])
                cs = slice((g + 1) * CHW, (g + 2) * CHW)
                nc.sync.dma_start(out=q2_t[:, cs], in_=q2[:, cs])
            nc.scalar.dma_start(out=k2_t[:, 2048:4096], in_=k2[:, 2048:4096])
            nc.scalar.dma_start(out=vpk_t[:, 1040:2080], in_=vpk[:, 1040:2080])
            nc.sync.dma_start(out=q2_t[:, 3 * CHW:NQ], in_=q2[:, 3 * CHW:NQ])

            # persistent P' for the symmetric (self) key tiles:
            # [kt 0..15][chunk 0..3][512 queries]
            # persistent P', chunk-major: strip (kt=a, chunk=c) at
            # column (c*16 + a)*512
            pbig = const.tile([128, NSELF * NCH * CHW], bf16, tag="pbig")
            # mirror buffer: 6 transpose units (source chunk c -> dest
            # chunk d, c < d), each [128, 2048] in xbar block-major order:
            # block m = i*4 + b_e  (i = dest qsub, b_e = dest kt - 4c)
            TIDX = {(0, 1): 0, (0, 2): 1, (0, 3): 2,
                    (1, 2): 3, (1, 3): 4, (2, 3): 5}
            pmir = const.tile([128, 6 * 4 * CHW], bf16, tag="pmir")
            pmv = pmir.rearrange("p (t i b w) -> p t i b w", t=6, i=4, b=4)

            # ---- PE + ACT warmup during the DMA prefix ----
            wz = const.tile([128, 512], bf16, tag="wz")
            nc.gpsimd.memset(wz, 0.0)
            wexp = const.tile([128, 1], f32, tag="wexp")
            nc.scalar.activation(wexp, wz[:, 0:1], EXP)  # pull exp table load
            wps = ps_s.tile([128, 1536], f32, tag="s")
            for _ in range(12):
                nc.tensor.matmul(wps[:, 0:512], wz[:, 0:128], wz,
                                 start=True, stop=True)

            # ---- main loop ----
            def pv(ch, kt, moving):
                nc.tensor.matmul(
                    o_accs[ch][:, :],
                    vpk_t[:, kt * 65:(kt + 1) * 65],
                    moving,
                    start=(pv_cnt[ch] == 0), stop=(pv_cnt[ch] == NKT - 1),
                    skip_group_check=True,
                )
                pv_cnt[ch] += 1

            o_accs = {}
            pv_cnt = {}
            prev = None          # (ch, kt0, nk, moving_ap)
            for ch in range(NCH):
                qs = slice(ch * CHW, (ch + 1) * CHW)
                o_accs[ch] = ps_o.tile([65, CHW], f32, tag="oacc",
                                       name=f"oacc{ch}")
                pv_cnt[ch] = 0
                su = _units(4 * ch if sym else 0, NSELF)
                fu = _units(NSELF, NKT)              # far strips
                units = []
                for i in range(max(len(su), len(fu))):
                    if i < len(su):
                        units.append((su[i], True))
                    if i < len(fu):
                        units.append((fu[i], False))
                asm = list(range(0, 4 * ch)) if sym else []
                for ui, ((kt0, nk), is_self) in enumerate(units):
                    s_u = ps_s.tile([128, nk * CHW], f32, tag="s")
                    for i in range(nk):
                        kt = kt0 + i
                        nc.tensor.matmul(
                            s_u[:, i * CHW:(i + 1) * CHW],
                            k2_t[:, kt * 128:(kt + 1) * 128],
                            q2_t[:, qs],
                            start=True, stop=True,
                        )
                    if prev is not None:
                        pch, pkt0, pnk, pmov = prev
                        for i in range(pnk):
                            pv(pch, pkt0 + i, pmov[i])
                        if pv_cnt[pch] == NKT:       # prev chunk complete
                            o_sb = sbo.tile([65, CHW], f32, tag="osb")
                            nc.vector.tensor_copy(o_sb, o_accs[pch])
                            ps = slice(pch * CHW, (pch + 1) * CHW)
                            nc.sync.dma_start(out=out[:, ps], in_=o_sb)
                    # spread transpose-assembled PVs over the early units
                    if ui >= 1:
                        for _ in range(2):
                            if asm:
                                a = asm.pop(0)
                                pv(ch, a, pmv[:, TIDX[(a // 4, ch)], :, a % 4])
                    # exp into Pbig (self) or a transient pool tile (far)
                    in_v = s_u[:, 0:nk * CHW].rearrange(
                        "p (a r) -> p a r", a=nk)
                    if is_self:
                        base = (ch * NSELF + kt0) * CHW
                        nc.scalar.activation(
                            pbig[:, base:base + nk * CHW].rearrange(
                                "p (a r) -> p a r", a=nk),
                            in_v, EXP)
                        mov = [pbig[:, base + i * CHW:base + (i + 1) * CHW]
                               for i in range(nk)]
                        if sym:
                            for i in range(nk):
                                a = kt0 + i
                                # once strips 4d..4d+3 of this chunk are
                                # all emitted, transpose them for chunk d
                                if a % 4 == 3 and a >= 4 * (ch + 1):
                                    d = a // 4
                                    t = TIDX[(ch, d)]
                                    sb = (ch * NSELF + 4 * d) * CHW
                                    nc.sync.dma_start(
                                        out=pmir[:, t * 2048:(t + 1) * 2048
                                                 ].rearrange(
                                            "p (m w) -> p m w", m=16),
                                        in_=pbig[:, sb:sb + 2048],
                                        transpose=True,
                                    )
                    else:
                        p_u = pp.tile([128, nk * CHW], bf16, tag="p")
                        nc.scalar.activation(
                            p_u.rearrange("p (a r) -> p a r", a=nk),
                            in_v, EXP)
                        mov = [p_u[:, i * CHW:(i + 1) * CHW]
                               for i in range(nk)]
                    prev = (ch, kt0, nk, mov)
                for a in asm:                        # leftover assembled PVs
                    pv(ch, a, pmv[:, TIDX[(a // 4, ch)], :, a % 4])
            # flush the final unit + last chunk output
            pch, pkt0, pnk, pmov = prev
            for i in range(pnk):
                pv(pch, pkt0 + i, pmov[i])
            o_sb = sbo.tile([65, CHW], f32, tag="osb")
            nc.vector.tensor_copy(o_sb, o_accs[pch])
            ps = slice(pch * CHW, (pch + 1) * CHW)
            # final output split across both HWDGE queues (Scalar is idle)
            nc.scalar.dma_start(out=out[0:33, ps], in_=o_sb[0:33, :])
            nc.sync.dma_start(out=out[33:65, ps], in_=o_sb[33:65, :])
    nc.compile()
    return nc


def _prep_inputs(x):
    """Host-side shard + operand marshaling. Returns list of 8 in_maps."""
    import ml_dtypes
    bf16 = ml_dtypes.bfloat16
    t = np.ascontiguousarray(x, np.float32).reshape(B_, NTOK, C_)
    in_maps = []
    for b in range(B_):
        t16 = t[b].astype(np.float16)               # [4096, 64]
        t32 = t16.astype(np.float32)
        gh = (0.5 * (t32.astype(np.float64) ** 2).sum(1)
              ).astype(np.float32).astype(np.float16)   # fp16 g rows
        gh32 = gh.astype(np.float32)
        G = np.float64(np.median(gh32))
        w = np.exp(gh32.astype(np.float64) - G).astype(np.float32)
        ones = np.ones(NTOK, np.float16)
        for h in range(2):
            sl = slice(h * NQ, (h + 1) * NQ)
            perm = np.r_[np.arange(h * NQ, (h + 1) * NQ),
                         np.arange((1 - h) * NQ, (2 - h) * NQ)]
            kp = t16[perm]                          # self keys first
            k2 = np.concatenate(
                [kp.T, ones[None, :], -gh[perm][None, :]]).astype(np.float16)
            wv = np.concatenate(
                [w[perm, None] * t32[perm], w[perm, None]],
                axis=1).astype(bf16)                # [4096, 65]
            vpk = np.concatenate(
                [wv[i * 128:(i + 1) * 128] for i in range(NKT)],
                axis=1).astype(bf16)                # [128, 32*65]
            q2 = np.concatenate(
                [t16[sl].T, -gh[sl][None, :], ones[None, sl]]
            ).astype(np.float16)                    # [66, 2048]
            in_maps.append({"q2": q2, "k2": k2, "vpk": vpk})
    return in_maps


def run(x, trace=False):
    from concourse.bass_utils import run_bass_kernel_spmd
    if "nc" not in _CACHE:
        _CACHE["nc"] = _build_nc(sym=os.environ.get("SYM", "1") == "1")
    nc = _CACHE["nc"]
    in_maps = _prep_inputs(x)
    res = run_bass_kernel_spmd(
        nc, in_maps, core_ids=list(range(NCORES)), trace=trace,
    )
    full = np.empty((B_, NTOK, C_), np.float32)
    for b in range(B_):
        for h in range(2):
            o = res.results[2 * b + h]["out"]        # [65, 2048]
            full[b, h * NQ:(h + 1) * NQ] = (o[0:C_] / o[C_]).T
    return full.reshape(B_, D_, H_, W_, C_), res


def kernel(x):
    out, _ = run(x, trace=False)
    return out
